# revision 31
# baseline (speedup 1.0000x reference)
"""Trainium2 Bass kernel for nn_MultiHeadAttention (B=8, D=512, N=2048, H=8).

Sharding: data-parallel over batch B — each of the 8 NeuronCores processes one
batch element end-to-end (no collectives).

Per-core pipeline (all matmuls fp16 operands, fp32 PSUM accumulation):
  1. V^T projection: vt[n, c] = sum_i x_v[i, n] * WvT[i, c] + bv  (PE)
  2. Q/K projections per head-pair m-tile, with RoPE applied as
     q = (Wq x + bq) * cos + (R @ (Wq x + bq)) * sin  where R is the
     rotate-half permutation matrix applied via an extra PE matmul.
  3. Attention per head-pair p and 512-wide n-chunk:
       S^T[m, n] = K_h^T-style matmul (head pair packed into PE row groups
       0-63 / 64-127 for 2x concurrency), exp via ScalarE (scale=1/8 fused),
       AV accumulated over m with a ones-column appended to V^T so row 64 of
       the output is the softmax denominator.
  4. Softmax normalization: denominators gathered via SBUF-SBUF DMA into an
     [8, 2048] tile, reciprocal on VectorE, broadcast across partitions via a
     rank-1 PE matmul, multiply on VectorE.
  5. Merge projection per head (K=64 matmuls) + bm bias, DMA out.

Channel permutation: reference reshape (B, D, N)->(B, 64, 8, N) maps original
channel d*8+h to head h, dim d. Weights are permuted on the host so on-device
channels are head-contiguous (c = h*64 + d).
"""

import os
import sys

for _p in ("/opt/trn_rl_repo", "/root/.axon_site/_ro/trn_rl_repo"):
    if os.path.isdir(_p) and _p not in sys.path:
        sys.path.append(_p)

import numpy as np

import concourse.bass as bass
import concourse.mybir as mybir
from concourse.tile import TileContext
from concourse.bass_utils import run_bass_kernel_spmd

F16 = mybir.dt.float16
F32 = mybir.dt.float32
AF = mybir.ActivationFunctionType
OP = mybir.AluOpType

B, D, N = 8, 512, 2048
H, DH = 8, 64
NCORES = 8
SCALE = 1.0 / (DH ** 0.5) / DH ** 0.0  # 1/sqrt(64) = 0.125


def _patch_tile_drain():
    """This walrus build rejects Drain instructions carrying >1 embedded sem
    waits ("Too many sync wait commands"). Redistribute the final TileContext
    drain's waits onto preceding SP nops, one per instruction. Idempotent."""
    if getattr(TileContext, "_ant_drain_patched", False):
        return
    orig = TileContext._drain_and_barrier

    from concourse.vector_clock import ScopedClock

    def patched(self, tick_clock, wait_clock):
        nops = [self.nc.sync.nop() for _ in range(32)]
        drain_inst = self.nc.sync.drain()
        wait_clock.add_sem_waits(
            drain_inst.ins, ScopedClock({None: tick_clock.global_clock})
        )
        si = drain_inst.ins.sync_info
        waits = list(si.on_wait) if si is not None and si.on_wait else []
        if len(waits) > 1:
            extra, keep = waits[:-1], waits[-1:]
            assert len(extra) <= len(nops)
            for w, nop in zip(extra, nops):
                nop.ins.sync_info = mybir.SyncInfo(on_wait=[w], on_update=[])
            drain_inst.ins.sync_info = mybir.SyncInfo(
                on_wait=keep,
                on_update=list(si.on_update) if si.on_update else [],
            )
        self.nc.all_engine_barrier()
        assert self.sems is not None
        popped = self.nc._tile_sem_poison_stack.pop()
        assert popped is self._sem_poison
        self.nc.clear_and_free_semaphores(list(self.sems.allocated().values()))
        self.nc.all_engine_barrier()

    # If the site copy of tile.py already splits waits (contains _spare_nops),
    # the monkeypatch is redundant but still correct: add_sem_waits on the
    # fresh drain yields the same wait set either way.
    import inspect

    src = None
    try:
        src = inspect.getsource(orig)
    except OSError:
        pass
    if src is None or "_spare_nops" not in src:
        TileContext._drain_and_barrier = patched
    TileContext._ant_drain_patched = True


def _split_sync_waits(nc, maxw=1):
    """This walrus build rejects instructions with more than one embedded sem
    wait ("Too many sync wait commands"). Move excess waits onto same-engine
    nops inserted immediately before the overloaded instruction."""
    n_split = 0
    for blk in nc.m.functions[0].blocks:
        insts = list(blk.instructions)
        out = []
        for inst in insts:
            si = inst.sync_info
            waits = list(si.on_wait) if si is not None and si.on_wait else []
            if len(waits) > maxw:
                extra, keep = waits[:-maxw], waits[-maxw:]
                for w in extra:
                    nop = mybir.InstNoOp(
                        name=nc.get_next_instruction_name(), ins=[], outs=[]
                    )
                    nop.engine = inst.engine
                    nop.sync_info = mybir.SyncInfo(on_wait=[w], on_update=[])
                    out.append(nop)
                inst.sync_info = mybir.SyncInfo(
                    on_wait=keep,
                    on_update=list(si.on_update) if si.on_update else [],
                )
                n_split += 1
            out.append(inst)
        blk.instructions = out
    return n_split


def _build_program():
    """Build the single-core Bass program (same program runs SPMD on 8 cores;
    per-core inputs differ)."""
    _patch_tile_drain()
    nc = bass.Bass()

    # ---- DRAM I/O ----
    xq_d = nc.dram_tensor("xq", [D, N], F16, kind="ExternalInput")
    xk_d = nc.dram_tensor("xk", [D, N], F16, kind="ExternalInput")
    xv_d = nc.dram_tensor("xv", [D, N], F16, kind="ExternalInput")
    wqT_d = nc.dram_tensor("wqT", [D, D], F16, kind="ExternalInput")
    wkT_d = nc.dram_tensor("wkT", [D, D], F16, kind="ExternalInput")
    wvT_d = nc.dram_tensor("wvT", [D, D], F16, kind="ExternalInput")
    wmT_d = nc.dram_tensor("wmT", [D, D], F16, kind="ExternalInput")
    rt_d = nc.dram_tensor("rt", [128, 128], F16, kind="ExternalInput")
    cos_d = nc.dram_tensor("cos2", [128, N], F16, kind="ExternalInput")
    sin_d = nc.dram_tensor("sin2", [128, N], F16, kind="ExternalInput")
    bq_d = nc.dram_tensor("bq", [128, 4], F32, kind="ExternalInput")
    bk_d = nc.dram_tensor("bk", [128, 4], F32, kind="ExternalInput")
    bm_d = nc.dram_tensor("bm", [128, 4], F32, kind="ExternalInput")
    bv_d = nc.dram_tensor("bv_row", [1, D], F16, kind="ExternalInput")
    out_d = nc.dram_tensor("out", [D, N], F32, kind="ExternalOutput")

    with TileContext(nc) as tc:
        with (
            tc.tile_pool(name="consts", bufs=1) as consts,
            tc.tile_pool(name="xin", bufs=1) as xin,
            tc.tile_pool(name="qk", bufs=1) as qkpool,
            tc.tile_pool(name="vt", bufs=1) as vtpool,
            tc.tile_pool(name="rope", bufs=3) as rope,
            tc.tile_pool(name="bcb", bufs=3) as bcbpool,
            tc.tile_pool(name="dram", bufs=1, space="DRAM") as drampool,
            tc.tile_pool(name="e", bufs=6) as epool,
            tc.tile_pool(name="attn", bufs=1) as attnpool,
            tc.tile_pool(name="stg", bufs=4) as stg,
            tc.tile_pool(name="osb", bufs=3) as osb,
            tc.tile_pool(name="ps", bufs=1, space="PSUM") as ps,
        ):
            # ---- constants to SBUF ----
            wq_sb = consts.tile([128, 4, D], F16, tag="wq")
            nc.sync.dma_start(out=wq_sb, in_=wqT_d.rearrange("(k p) m -> p k m", p=128))
            wk_sb = consts.tile([128, 4, D], F16, tag="wk")
            nc.sync.dma_start(out=wk_sb, in_=wkT_d.rearrange("(k p) m -> p k m", p=128))
            wv_sb = consts.tile([128, 4, D], F16, tag="wv")
            nc.sync.dma_start(out=wv_sb, in_=wvT_d.rearrange("(k p) m -> p k m", p=128))
            wm_sb = consts.tile([128, 4, D], F16, tag="wm")
            nc.sync.dma_start(out=wm_sb, in_=wmT_d.rearrange("(g c) m -> c g m", c=128))
            rt_sb = consts.tile([128, 128], F16, tag="rt")
            nc.sync.dma_start(out=rt_sb, in_=rt_d[:, :])
            cos_sb = consts.tile([128, N], F16, tag="cos")
            nc.sync.dma_start(out=cos_sb, in_=cos_d[:, :])
            sin_sb = consts.tile([128, N], F16, tag="sin")
            nc.sync.dma_start(out=sin_sb, in_=sin_d[:, :])
            bq_sb = consts.tile([128, 4], F32, tag="bq")
            nc.sync.dma_start(out=bq_sb, in_=bq_d[:, :])
            bk_sb = consts.tile([128, 4], F32, tag="bk")
            nc.sync.dma_start(out=bk_sb, in_=bk_d[:, :])
            bm_sb = consts.tile([128, 4], F32, tag="bm")
            nc.sync.dma_start(out=bm_sb, in_=bm_d[:, :])
            bvb_sb = consts.tile([128, D], F16, tag="bvb")
            nc.sync.dma_start(out=bvb_sb, in_=bv_d[:, :].to_broadcast([128, D]))
            ones_sb = consts.tile([1, 128], F16, tag="ones")
            nc.vector.memset(ones_sb, 1.0)
            denoms = consts.tile([H, N], F16, tag="denoms")
            recip = consts.tile([H, N], F16, tag="recip")
            recip_dram = drampool.tile([H, N], F16, tag="recip_dram")

            # ---- load inputs (per-k-tile DMAs so consumers start early) ----
            xv_sb = xin.tile([128, 4, N], F16, tag="xv")
            xq_sb = xin.tile([128, 4, N], F16, tag="xq")
            xk_sb = xin.tile([128, 4, N], F16, tag="xk")
            for kk in range(4):
                ks = slice(kk * 128, (kk + 1) * 128)
                nc.sync.dma_start(out=xv_sb[:, kk, :], in_=xv_d[ks, :])
            for kk in range(4):
                ks = slice(kk * 128, (kk + 1) * 128)
                nc.sync.dma_start(out=xq_sb[:, kk, :], in_=xq_d[ks, :])
            for kk in range(4):
                ks = slice(kk * 128, (kk + 1) * 128)
                nc.sync.dma_start(out=xk_sb[:, kk, :], in_=xk_d[ks, :])

            # V^T (+ ones column at index 64 per head) and Q/K destinations
            vt_sb = vtpool.tile([128, 16, H, 65], F16, tag="vt")
            nc.vector.memset(vt_sb[:, :, :, 64:65], 1.0)
            q_sb = qkpool.tile([128, 4, N], F16, tag="q")
            k_sb = qkpool.tile([128, 4, N], F16, tag="k")
            # per chunk ci=(pair, nc): head A on partitions 0-63, head B on
            # 64-127 (B arrives via cross-partition SBUF DMA) -> full-array merge
            attn2_sb = attnpool.tile([128, 16, 512], F16, tag="attn2")

            # ---- 1. V^T projection ----
            for nt in range(16):
                vps = ps.tile([128, D], F32, tag="st", bufs=3)
                for kk in range(4):
                    nc.tensor.matmul(
                        vps,
                        lhsT=xv_sb[:, kk, nt * 128:(nt + 1) * 128],
                        rhs=wv_sb[:, kk, :],
                        start=(kk == 0),
                        stop=(kk == 3),
                    )
                nc.vector.tensor_tensor(
                    out=vt_sb[:, nt, :, 0:64],
                    in0=vps.rearrange("p (h d) -> p h d", h=H),
                    in1=bvb_sb.rearrange("p (h d) -> p h d", h=H),
                    op=OP.add,
                )

            # ---- 2. Q/K projection + RoPE for one head pair ----
            def project_pair(p):
                ms = slice(p * 128, (p + 1) * 128)
                for dst, x_sb, w_sb, b_sb in (
                    (q_sb, xq_sb, wq_sb, bq_sb),
                    (k_sb, xk_sb, wk_sb, bk_sb),
                ):
                    raw = rope.tile([128, N], F16, tag="rope", name="raw")
                    for nh in range(2):
                        nhs = slice(nh * 1024, (nh + 1) * 1024)
                        pps = ps.tile([128, 1024], F32, tag="st", bufs=3, name="pps")
                        for n4 in range(2):
                            ns = slice(n4 * 512, (n4 + 1) * 512)
                            for kk in range(4):
                                nc.tensor.matmul(
                                    pps[:, ns],
                                    lhsT=w_sb[:, kk, ms],
                                    rhs=x_sb[:, kk, nh * 1024 + n4 * 512:
                                             nh * 1024 + (n4 + 1) * 512],
                                    start=(kk == 0),
                                    stop=(kk == 3),
                                )
                        nc.scalar.activation(
                            out=raw[:, nhs], in_=pps, func=AF.Identity,
                            bias=b_sb[:, p:p + 1],
                        )
                    rot = rope.tile([128, N], F16, tag="rope", name="rot")
                    for nh in range(2):
                        nhs = slice(nh * 1024, (nh + 1) * 1024)
                        rps = ps.tile([128, 1024], F32, tag="st", bufs=3, name="rps")
                        nc.tensor.matmul(
                            rps[:, 0:512], lhsT=rt_sb,
                            rhs=raw[:, nh * 1024:nh * 1024 + 512],
                        )
                        nc.tensor.matmul(
                            rps[:, 512:1024], lhsT=rt_sb,
                            rhs=raw[:, nh * 1024 + 512:(nh + 1) * 1024],
                        )
                        nc.vector.tensor_copy(out=rot[:, nhs], in_=rps)
                    t1 = rope.tile([128, N], F16, tag="rope", name="t1")
                    nc.vector.tensor_mul(t1, raw, cos_sb)
                    nc.vector.tensor_mul(rot, rot, sin_sb)
                    nc.vector.tensor_add(dst[:, p, :], t1, rot)

            # ---- 3. attention for one (n-chunk, head-pair) ----
            # Software-pipelined m-loop: per iteration trace exp(m), then
            # QK(m+2), then AV(m-1). With 3 st buffers, QK(m+2)'s slot was
            # freed by exp(m-1), and AV(m-1)'s exp finished an iteration ago
            # — so the PE FIFO never stalls on ScalarE and ScalarE's exps run
            # back-to-back (it is the bottleneck engine in this phase).
            def attn_chunk(nc4, p):
                ns = slice(nc4 * 512, (nc4 + 1) * 512)
                ci = p * 4 + nc4
                av = ps.tile([65, 1024], F32, tag="av", name="av")

                def qk(m):
                    mt = slice(m * 128, (m + 1) * 128)
                    st = ps.tile([128, 1024], F32, tag="st", bufs=3, name="st")
                    nc.tensor.matmul(
                        st[:, 0:512],
                        lhsT=k_sb[0:64, p, mt], rhs=q_sb[0:64, p, ns],
                    )
                    nc.tensor.matmul(
                        st[:, 512:1024],
                        lhsT=k_sb[64:128, p, mt], rhs=q_sb[64:128, p, ns],
                    )
                    return st

                def avmm(m, e):
                    nc.tensor.matmul(
                        av[:, 0:512],
                        lhsT=vt_sb[:, m, 2 * p, :], rhs=e[:, 0:512],
                        start=(m == 0), stop=(m == 15),
                    )
                    nc.tensor.matmul(
                        av[:, 512:1024],
                        lhsT=vt_sb[:, m, 2 * p + 1, :], rhs=e[:, 512:1024],
                        start=(m == 0), stop=(m == 15),
                    )

                sts = {0: qk(0), 1: qk(1)}
                es = {}
                for m in range(16):
                    e = epool.tile([128, 1024], F16, tag="e", name="e")
                    nc.scalar.activation(
                        out=e, in_=sts.pop(m), func=AF.Exp, scale=SCALE
                    )
                    es[m] = e
                    if m + 2 <= 15:
                        sts[m + 2] = qk(m + 2)
                    if m >= 1:
                        avmm(m - 1, es.pop(m - 1))
                avmm(15, es.pop(15))
                astage = stg.tile([65, 512], F16, tag="stg", name="astage")
                bstage = stg.tile([65, 512], F16, tag="stg", name="bstage")
                nc.vector.tensor_copy(out=astage, in_=av[:, 0:512])
                nc.vector.tensor_copy(out=bstage, in_=av[:, 512:1024])
                nc.sync.dma_start(out=attn2_sb[0:64, ci, :], in_=astage[0:64, :])
                nc.sync.dma_start(out=attn2_sb[64:128, ci, :], in_=bstage[0:64, :])
                nc.sync.dma_start(
                    out=denoms[2 * p:2 * p + 1, ns], in_=astage[64:65, :]
                )
                nc.sync.dma_start(
                    out=denoms[2 * p + 1:2 * p + 2, ns], in_=bstage[64:65, :]
                )

            # ---- 4+5. softmax normalization + merge for one n-chunk ----
            def norm_chunk(nc4):
                # DVE + DMA only — keeps the PE FIFO free for attention
                ns = slice(nc4 * 512, (nc4 + 1) * 512)
                with nc.allow_low_precision(reason="fp16 softmax denominators"):
                    nc.vector.reciprocal(out=recip[:, ns], in_=denoms[:, ns])
                nc.sync.dma_start(out=recip_dram[:, ns], in_=recip[:, ns])
                for p in range(4):
                    ci = p * 4 + nc4
                    bcb = bcbpool.tile([128, 512], F16, tag="bcb", name="bcb")
                    nc.sync.dma_start(
                        out=bcb[0:64, :],
                        in_=recip_dram[2 * p:2 * p + 1, ns].to_broadcast([64, 512]),
                    )
                    nc.sync.dma_start(
                        out=bcb[64:128, :],
                        in_=recip_dram[2 * p + 1:2 * p + 2, ns].to_broadcast([64, 512]),
                    )
                    nc.vector.tensor_tensor(
                        out=attn2_sb[:, ci, :],
                        in0=attn2_sb[:, ci, :], in1=bcb, op=OP.mult,
                    )

            def merge_mo(nc4, mo):
                ns = slice(nc4 * 512, (nc4 + 1) * 512)
                mos = slice(mo * 128, (mo + 1) * 128)
                mps = ps.tile([128, 512], F32, tag="st", bufs=3, name="mps")
                for g in range(4):
                    ci = g * 4 + nc4
                    nc.tensor.matmul(
                        mps,
                        lhsT=wm_sb[:, g, mos],
                        rhs=attn2_sb[:, ci, :],
                        start=(g == 0), stop=(g == 3),
                    )
                o = osb.tile([128, 512], F32, tag="o", name="o")
                nc.scalar.activation(
                    out=o, in_=mps, func=AF.Identity, bias=bm_sb[:, mo:mo + 1]
                )
                nc.sync.dma_start(out=out_d[mos, ns], in_=o)

            # trace order: interleave the first n-chunk's attention with the
            # projections so ScalarE starts exp work as early as possible, and
            # delay each chunk's normalize+merge until after the NEXT chunk's
            # attention is queued — its long dependency chain (denom DMA →
            # reciprocal → rrow DMA → bcast → multiply) then resolves while
            # the PE works on attention instead of stalling its FIFO.
            # Global schedule: projections interleave with chunks of already-
            # projected pairs so ScalarE never starves; each chunk's
            # normalize (DVE/DMA-only) fires once its 4 pairs are done, and
            # merge matmuls spread between later attention chunks.
            sched = [
                "p0", "A00", "p1", "A01", "A10", "A11", "A02", "p2",
                "A12", "A03", "A20", "A13", "A21", "p3", "A22", "A30",
                "n0", "m00", "A31", "m01", "m02", "A23", "m03", "n1",
                "A32", "m10", "m11", "A33", "n2", "m20", "m21", "m12",
                "m13", "m22", "m23", "n3", "m30", "m31", "m32", "m33",
            ]
            for step in sched:
                kind, a = step[0], step[1:]
                if kind == "p":
                    project_pair(int(a))
                elif kind == "A":
                    attn_chunk(int(a[1]), int(a[0]))
                elif kind == "n":
                    norm_chunk(int(a))
                elif kind == "m":
                    merge_mo(int(a[0]), int(a[1]))

    _split_sync_waits(nc)
    return nc


_NC_CACHE = None
LAST_RESULT = None


def _get_nc():
    global _NC_CACHE
    if _NC_CACHE is None:
        _NC_CACHE = _build_program()
    return _NC_CACHE


def _host_prep(query, key, value, enc_cos, enc_sin, wq, bq, wk, bk, wv, bv, wm, bm):
    """Host-side weight permutation / transposition / dtype casts."""
    c = np.arange(D)
    perm = (c % DH) * H + (c // DH)  # new head-contiguous channel -> original

    wq_r, wk_r, wv_r = wq[perm], wk[perm], wv[perm]
    bq_r, bk_r, bv_r = bq[perm], bk[perm], bv[perm]
    wm_r = wm[:, perm]

    r64 = np.zeros((DH, DH), np.float32)
    idx = np.arange(DH // 2)
    r64[2 * idx, 2 * idx + 1] = -1.0
    r64[2 * idx + 1, 2 * idx] = 1.0
    r128 = np.zeros((128, 128), np.float32)
    r128[0:64, 0:64] = r64
    r128[64:128, 64:128] = r64

    cos64 = enc_cos[0, :, 0, :]
    sin64 = enc_sin[0, :, 0, :]

    shared = {
        "wqT": np.ascontiguousarray(wq_r.T).astype(np.float16),
        "wkT": np.ascontiguousarray(wk_r.T).astype(np.float16),
        "wvT": np.ascontiguousarray(wv_r.T).astype(np.float16),
        "wmT": np.ascontiguousarray(wm_r.T).astype(np.float16),
        "rt": np.ascontiguousarray(r128.T).astype(np.float16),
        "cos2": np.vstack([cos64, cos64]).astype(np.float16),
        "sin2": np.vstack([sin64, sin64]).astype(np.float16),
        "bq": np.ascontiguousarray(bq_r.reshape(4, 128).T).astype(np.float32),
        "bk": np.ascontiguousarray(bk_r.reshape(4, 128).T).astype(np.float32),
        "bm": np.ascontiguousarray(bm.reshape(4, 128).T).astype(np.float32),
        "bv_row": bv_r[None, :].astype(np.float16),
    }
    q16 = np.asarray(query, np.float16)
    k16 = np.asarray(key, np.float16)
    v16 = np.asarray(value, np.float16)
    in_maps = []
    for b in range(B):
        m = dict(shared)
        m["xq"] = np.ascontiguousarray(q16[b])
        m["xk"] = np.ascontiguousarray(k16[b])
        m["xv"] = np.ascontiguousarray(v16[b])
        in_maps.append(m)
    return in_maps


def kernel(query, key, value, enc_cos, enc_sin, wq, bq, wk, bk, wv, bv, wm, bm):
    global LAST_RESULT
    query = np.asarray(query, np.float32)
    key = np.asarray(key, np.float32)
    value = np.asarray(value, np.float32)
    in_maps = _host_prep(
        query, key, value,
        np.asarray(enc_cos, np.float32), np.asarray(enc_sin, np.float32),
        np.asarray(wq, np.float32), np.asarray(bq, np.float32),
        np.asarray(wk, np.float32), np.asarray(bk, np.float32),
        np.asarray(wv, np.float32), np.asarray(bv, np.float32),
        np.asarray(wm, np.float32), np.asarray(bm, np.float32),
    )
    nc = _get_nc()
    res = run_bass_kernel_spmd(nc, in_maps, core_ids=list(range(NCORES)))
    LAST_RESULT = res
    out = np.stack([res.results[b]["out"] for b in range(B)], axis=0)
    return out.astype(np.float32)


# revision 32
# speedup vs baseline: 1.0425x; 1.0425x over previous
"""Trainium2 Bass kernel for nn_MultiHeadAttention (B=8, D=512, N=2048, H=8).

Sharding: data-parallel over batch B — each of the 8 NeuronCores processes one
batch element end-to-end (no collectives).

Per-core pipeline (all matmuls fp16 operands, fp32 PSUM accumulation):
  1. V^T projection: vt[n, c] = sum_i x_v[i, n] * WvT[i, c] + bv  (PE)
  2. Q/K projections per head-pair m-tile, with RoPE applied as
     q = (Wq x + bq) * cos + (R @ (Wq x + bq)) * sin  where R is the
     rotate-half permutation matrix applied via an extra PE matmul.
  3. Attention per head-pair p and 512-wide n-chunk:
       S^T[m, n] = K_h^T-style matmul (head pair packed into PE row groups
       0-63 / 64-127 for 2x concurrency), exp via ScalarE (scale=1/8 fused),
       AV accumulated over m with a ones-column appended to V^T so row 64 of
       the output is the softmax denominator.
  4. Softmax normalization: denominators gathered via SBUF-SBUF DMA into an
     [8, 2048] tile, reciprocal on VectorE, broadcast across partitions via a
     rank-1 PE matmul, multiply on VectorE.
  5. Merge projection per head (K=64 matmuls) + bm bias, DMA out.

Channel permutation: reference reshape (B, D, N)->(B, 64, 8, N) maps original
channel d*8+h to head h, dim d. Weights are permuted on the host so on-device
channels are head-contiguous (c = h*64 + d).
"""

import os
import sys

for _p in ("/opt/trn_rl_repo", "/root/.axon_site/_ro/trn_rl_repo"):
    if os.path.isdir(_p) and _p not in sys.path:
        sys.path.append(_p)

import numpy as np

import concourse.bass as bass
import concourse.mybir as mybir
from concourse.tile import TileContext
from concourse.bass_utils import run_bass_kernel_spmd

F16 = mybir.dt.float16
F32 = mybir.dt.float32
AF = mybir.ActivationFunctionType
OP = mybir.AluOpType

B, D, N = 8, 512, 2048
H, DH = 8, 64
NCORES = 8
SCALE = 1.0 / (DH ** 0.5) / DH ** 0.0  # 1/sqrt(64) = 0.125


def _patch_tile_drain():
    """This walrus build rejects Drain instructions carrying >1 embedded sem
    waits ("Too many sync wait commands"). Redistribute the final TileContext
    drain's waits onto preceding SP nops, one per instruction. Idempotent."""
    if getattr(TileContext, "_ant_drain_patched", False):
        return
    orig = TileContext._drain_and_barrier

    from concourse.vector_clock import ScopedClock

    def patched(self, tick_clock, wait_clock):
        nops = [self.nc.sync.nop() for _ in range(32)]
        drain_inst = self.nc.sync.drain()
        wait_clock.add_sem_waits(
            drain_inst.ins, ScopedClock({None: tick_clock.global_clock})
        )
        si = drain_inst.ins.sync_info
        waits = list(si.on_wait) if si is not None and si.on_wait else []
        if len(waits) > 1:
            extra, keep = waits[:-1], waits[-1:]
            assert len(extra) <= len(nops)
            for w, nop in zip(extra, nops):
                nop.ins.sync_info = mybir.SyncInfo(on_wait=[w], on_update=[])
            drain_inst.ins.sync_info = mybir.SyncInfo(
                on_wait=keep,
                on_update=list(si.on_update) if si.on_update else [],
            )
        self.nc.all_engine_barrier()
        assert self.sems is not None
        popped = self.nc._tile_sem_poison_stack.pop()
        assert popped is self._sem_poison
        self.nc.clear_and_free_semaphores(list(self.sems.allocated().values()))
        self.nc.all_engine_barrier()

    # If the site copy of tile.py already splits waits (contains _spare_nops),
    # the monkeypatch is redundant but still correct: add_sem_waits on the
    # fresh drain yields the same wait set either way.
    import inspect

    src = None
    try:
        src = inspect.getsource(orig)
    except OSError:
        pass
    if src is None or "_spare_nops" not in src:
        TileContext._drain_and_barrier = patched
    TileContext._ant_drain_patched = True


def _split_sync_waits(nc, maxw=1):
    """This walrus build rejects instructions with more than one embedded sem
    wait ("Too many sync wait commands"). Move excess waits onto same-engine
    nops inserted immediately before the overloaded instruction."""
    n_split = 0
    for blk in nc.m.functions[0].blocks:
        insts = list(blk.instructions)
        out = []
        for inst in insts:
            si = inst.sync_info
            waits = list(si.on_wait) if si is not None and si.on_wait else []
            if len(waits) > maxw:
                extra, keep = waits[:-maxw], waits[-maxw:]
                for w in extra:
                    nop = mybir.InstNoOp(
                        name=nc.get_next_instruction_name(), ins=[], outs=[]
                    )
                    nop.engine = inst.engine
                    nop.sync_info = mybir.SyncInfo(on_wait=[w], on_update=[])
                    out.append(nop)
                inst.sync_info = mybir.SyncInfo(
                    on_wait=keep,
                    on_update=list(si.on_update) if si.on_update else [],
                )
                n_split += 1
            out.append(inst)
        blk.instructions = out
    return n_split


def _build_program():
    """Build the single-core Bass program (same program runs SPMD on 8 cores;
    per-core inputs differ)."""
    _patch_tile_drain()
    nc = bass.Bass()

    # ---- DRAM I/O ----
    xq_d = nc.dram_tensor("xq", [D, N], F16, kind="ExternalInput")
    xk_d = nc.dram_tensor("xk", [D, N], F16, kind="ExternalInput")
    xv_d = nc.dram_tensor("xv", [D, N], F16, kind="ExternalInput")
    wqT_d = nc.dram_tensor("wqT", [D, D], F16, kind="ExternalInput")
    wkT_d = nc.dram_tensor("wkT", [D, D], F16, kind="ExternalInput")
    wvT_d = nc.dram_tensor("wvT", [D, D], F16, kind="ExternalInput")
    wmT_d = nc.dram_tensor("wmT", [D, D], F16, kind="ExternalInput")
    rt_d = nc.dram_tensor("rt", [128, 128], F16, kind="ExternalInput")
    cos_d = nc.dram_tensor("cos2", [128, N], F16, kind="ExternalInput")
    sin_d = nc.dram_tensor("sin2", [128, N], F16, kind="ExternalInput")
    bq_d = nc.dram_tensor("bq", [128, 4], F32, kind="ExternalInput")
    bk_d = nc.dram_tensor("bk", [128, 4], F32, kind="ExternalInput")
    bm_d = nc.dram_tensor("bm", [128, 4], F32, kind="ExternalInput")
    bv_d = nc.dram_tensor("bv_row", [1, D], F16, kind="ExternalInput")
    out_d = nc.dram_tensor("out", [D, N], F32, kind="ExternalOutput")

    with TileContext(nc) as tc:
        with (
            tc.tile_pool(name="consts", bufs=1) as consts,
            tc.tile_pool(name="xin", bufs=1) as xin,
            tc.tile_pool(name="qk", bufs=1) as qkpool,
            tc.tile_pool(name="vt", bufs=1) as vtpool,
            tc.tile_pool(name="rope", bufs=3) as rope,
            tc.tile_pool(name="bcb", bufs=3) as bcbpool,
            tc.tile_pool(name="dram", bufs=1, space="DRAM") as drampool,
            tc.tile_pool(name="e", bufs=8) as epool,
            tc.tile_pool(name="attn", bufs=1) as attnpool,
            tc.tile_pool(name="stg", bufs=6) as stg,
            tc.tile_pool(name="osb", bufs=3) as osb,
            tc.tile_pool(name="ps", bufs=1, space="PSUM") as ps,
        ):
            # ---- constants to SBUF ----
            wq_sb = consts.tile([128, 4, D], F16, tag="wq")
            nc.sync.dma_start(out=wq_sb, in_=wqT_d.rearrange("(k p) m -> p k m", p=128))
            wk_sb = consts.tile([128, 4, D], F16, tag="wk")
            nc.sync.dma_start(out=wk_sb, in_=wkT_d.rearrange("(k p) m -> p k m", p=128))
            wv_sb = consts.tile([128, 4, D], F16, tag="wv")
            nc.sync.dma_start(out=wv_sb, in_=wvT_d.rearrange("(k p) m -> p k m", p=128))
            wm_sb = consts.tile([128, 4, D], F16, tag="wm")
            nc.sync.dma_start(out=wm_sb, in_=wmT_d.rearrange("(g c) m -> c g m", c=128))
            rt_sb = consts.tile([128, 128], F16, tag="rt")
            nc.sync.dma_start(out=rt_sb, in_=rt_d[:, :])
            cos_sb = consts.tile([128, N], F16, tag="cos")
            nc.sync.dma_start(out=cos_sb, in_=cos_d[:, :])
            sin_sb = consts.tile([128, N], F16, tag="sin")
            nc.sync.dma_start(out=sin_sb, in_=sin_d[:, :])
            bq_sb = consts.tile([128, 4], F32, tag="bq")
            nc.sync.dma_start(out=bq_sb, in_=bq_d[:, :])
            bk_sb = consts.tile([128, 4], F32, tag="bk")
            nc.sync.dma_start(out=bk_sb, in_=bk_d[:, :])
            bm_sb = consts.tile([128, 4], F32, tag="bm")
            nc.sync.dma_start(out=bm_sb, in_=bm_d[:, :])
            bvb_sb = consts.tile([128, D], F16, tag="bvb")
            nc.sync.dma_start(out=bvb_sb, in_=bv_d[:, :].to_broadcast([128, D]))
            ones_sb = consts.tile([1, 128], F16, tag="ones")
            nc.vector.memset(ones_sb, 1.0)
            denoms = consts.tile([H, N], F16, tag="denoms")
            recip = consts.tile([H, N], F16, tag="recip")
            recip_dram = drampool.tile([H, N], F16, tag="recip_dram")

            # ---- load inputs (per-k-tile DMAs so consumers start early) ----
            xv_sb = xin.tile([128, 4, N], F16, tag="xv")
            xq_sb = xin.tile([128, 4, N], F16, tag="xq")
            xk_sb = xin.tile([128, 4, N], F16, tag="xk")
            for kk in range(4):
                ks = slice(kk * 128, (kk + 1) * 128)
                nc.sync.dma_start(out=xv_sb[:, kk, :], in_=xv_d[ks, :])
            for kk in range(4):
                ks = slice(kk * 128, (kk + 1) * 128)
                nc.sync.dma_start(out=xq_sb[:, kk, :], in_=xq_d[ks, :])
            for kk in range(4):
                ks = slice(kk * 128, (kk + 1) * 128)
                nc.sync.dma_start(out=xk_sb[:, kk, :], in_=xk_d[ks, :])

            # V^T (+ ones column at index 64 per head) and Q/K destinations
            vt_sb = vtpool.tile([128, 16, H, 65], F16, tag="vt")
            nc.vector.memset(vt_sb[:, :, :, 64:65], 1.0)
            q_sb = qkpool.tile([128, 4, N], F16, tag="q")
            k_sb = qkpool.tile([128, 4, N], F16, tag="k")
            # per chunk ci=(pair, nc): head A on partitions 0-63, head B on
            # 64-127 (B arrives via cross-partition SBUF DMA) -> full-array merge
            attn2_sb = attnpool.tile([128, 16, 512], F16, tag="attn2")

            # ---- 1. V^T projection ----
            for nt in range(16):
                vps = ps.tile([128, D], F32, tag="st", bufs=3)
                for kk in range(4):
                    nc.tensor.matmul(
                        vps,
                        lhsT=xv_sb[:, kk, nt * 128:(nt + 1) * 128],
                        rhs=wv_sb[:, kk, :],
                        start=(kk == 0),
                        stop=(kk == 3),
                    )
                nc.vector.tensor_tensor(
                    out=vt_sb[:, nt, :, 0:64],
                    in0=vps.rearrange("p (h d) -> p h d", h=H),
                    in1=bvb_sb.rearrange("p (h d) -> p h d", h=H),
                    op=OP.add,
                )

            # ---- 2. Q/K projection + RoPE for one head pair ----
            def project_pair(p):
                ms = slice(p * 128, (p + 1) * 128)
                for dst, x_sb, w_sb, b_sb in (
                    (q_sb, xq_sb, wq_sb, bq_sb),
                    (k_sb, xk_sb, wk_sb, bk_sb),
                ):
                    raw = rope.tile([128, N], F16, tag="rope", name="raw")
                    for nh in range(2):
                        nhs = slice(nh * 1024, (nh + 1) * 1024)
                        pps = ps.tile([128, 1024], F32, tag="st", bufs=3, name="pps")
                        for n4 in range(2):
                            ns = slice(n4 * 512, (n4 + 1) * 512)
                            for kk in range(4):
                                nc.tensor.matmul(
                                    pps[:, ns],
                                    lhsT=w_sb[:, kk, ms],
                                    rhs=x_sb[:, kk, nh * 1024 + n4 * 512:
                                             nh * 1024 + (n4 + 1) * 512],
                                    start=(kk == 0),
                                    stop=(kk == 3),
                                )
                        nc.scalar.activation(
                            out=raw[:, nhs], in_=pps, func=AF.Identity,
                            bias=b_sb[:, p:p + 1],
                        )
                    rot = rope.tile([128, N], F16, tag="rope", name="rot")
                    for nh in range(2):
                        nhs = slice(nh * 1024, (nh + 1) * 1024)
                        rps = ps.tile([128, 1024], F32, tag="st", bufs=3, name="rps")
                        nc.tensor.matmul(
                            rps[:, 0:512], lhsT=rt_sb,
                            rhs=raw[:, nh * 1024:nh * 1024 + 512],
                        )
                        nc.tensor.matmul(
                            rps[:, 512:1024], lhsT=rt_sb,
                            rhs=raw[:, nh * 1024 + 512:(nh + 1) * 1024],
                        )
                        nc.vector.tensor_copy(out=rot[:, nhs], in_=rps)
                    t1 = rope.tile([128, N], F16, tag="rope", name="t1")
                    nc.vector.tensor_mul(t1, raw, cos_sb)
                    nc.vector.tensor_mul(rot, rot, sin_sb)
                    nc.vector.tensor_add(dst[:, p, :], t1, rot)

            # ---- 3. attention for one (n-chunk, head-pair) ----
            # Software-pipelined m-loop: per iteration trace exp(m), then
            # QK(m+2), then AV(m-1). With 3 st buffers, QK(m+2)'s slot was
            # freed by exp(m-1), and AV(m-1)'s exp finished an iteration ago
            # — so the PE FIFO never stalls on ScalarE and ScalarE's exps run
            # back-to-back (it is the bottleneck engine in this phase).
            def attn_chunk(nc4, p):
                ns = slice(nc4 * 512, (nc4 + 1) * 512)
                ci = p * 4 + nc4
                av = ps.tile([65, 1024], F32, tag="av", name="av")

                def qk(m):
                    mt = slice(m * 128, (m + 1) * 128)
                    st = ps.tile([128, 1024], F32, tag="st", bufs=3, name="st")
                    nc.tensor.matmul(
                        st[:, 0:512],
                        lhsT=k_sb[0:64, p, mt], rhs=q_sb[0:64, p, ns],
                    )
                    nc.tensor.matmul(
                        st[:, 512:1024],
                        lhsT=k_sb[64:128, p, mt], rhs=q_sb[64:128, p, ns],
                    )
                    return st

                def avmm(m, e):
                    nc.tensor.matmul(
                        av[:, 0:512],
                        lhsT=vt_sb[:, m, 2 * p, :], rhs=e[:, 0:512],
                        start=(m == 0), stop=(m == 15),
                    )
                    nc.tensor.matmul(
                        av[:, 512:1024],
                        lhsT=vt_sb[:, m, 2 * p + 1, :], rhs=e[:, 512:1024],
                        start=(m == 0), stop=(m == 15),
                    )

                sts = {0: qk(0), 1: qk(1)}
                es = {}
                for m in range(16):
                    e = epool.tile([128, 1024], F16, tag="e", name="e")
                    nc.scalar.activation(
                        out=e, in_=sts.pop(m), func=AF.Exp, scale=SCALE
                    )
                    es[m] = e
                    if m + 2 <= 15:
                        sts[m + 2] = qk(m + 2)
                    if m >= 1:
                        avmm(m - 1, es.pop(m - 1))
                avmm(15, es.pop(15))
                astage = stg.tile([65, 512], F16, tag="stg", name="astage")
                bstage = stg.tile([65, 512], F16, tag="stg", name="bstage")
                nc.vector.tensor_copy(out=astage, in_=av[:, 0:512])
                nc.vector.tensor_copy(out=bstage, in_=av[:, 512:1024])
                nc.sync.dma_start(out=attn2_sb[0:64, ci, :], in_=astage[0:64, :])
                nc.sync.dma_start(out=attn2_sb[64:128, ci, :], in_=bstage[0:64, :])
                nc.sync.dma_start(
                    out=denoms[2 * p:2 * p + 1, ns], in_=astage[64:65, :]
                )
                nc.sync.dma_start(
                    out=denoms[2 * p + 1:2 * p + 2, ns], in_=bstage[64:65, :]
                )

            # ---- 4+5. softmax normalization + merge for one n-chunk ----
            def norm_chunk(nc4):
                # DVE + DMA only — keeps the PE FIFO free for attention
                ns = slice(nc4 * 512, (nc4 + 1) * 512)
                with nc.allow_low_precision(reason="fp16 softmax denominators"):
                    nc.vector.reciprocal(out=recip[:, ns], in_=denoms[:, ns])
                nc.sync.dma_start(out=recip_dram[:, ns], in_=recip[:, ns])
                for p in range(4):
                    ci = p * 4 + nc4
                    bcb = bcbpool.tile([128, 512], F16, tag="bcb", name="bcb")
                    nc.sync.dma_start(
                        out=bcb[0:64, :],
                        in_=recip_dram[2 * p:2 * p + 1, ns].to_broadcast([64, 512]),
                    )
                    nc.sync.dma_start(
                        out=bcb[64:128, :],
                        in_=recip_dram[2 * p + 1:2 * p + 2, ns].to_broadcast([64, 512]),
                    )
                    nc.vector.tensor_tensor(
                        out=attn2_sb[:, ci, :],
                        in0=attn2_sb[:, ci, :], in1=bcb, op=OP.mult,
                    )

            def merge_mo(nc4, mo):
                ns = slice(nc4 * 512, (nc4 + 1) * 512)
                mos = slice(mo * 128, (mo + 1) * 128)
                mps = ps.tile([128, 512], F32, tag="st", bufs=3, name="mps")
                for g in range(4):
                    ci = g * 4 + nc4
                    nc.tensor.matmul(
                        mps,
                        lhsT=wm_sb[:, g, mos],
                        rhs=attn2_sb[:, ci, :],
                        start=(g == 0), stop=(g == 3),
                    )
                o = osb.tile([128, 512], F32, tag="o", name="o")
                nc.scalar.activation(
                    out=o, in_=mps, func=AF.Identity, bias=bm_sb[:, mo:mo + 1]
                )
                nc.sync.dma_start(out=out_d[mos, ns], in_=o)

            # trace order: interleave the first n-chunk's attention with the
            # projections so ScalarE starts exp work as early as possible, and
            # delay each chunk's normalize+merge until after the NEXT chunk's
            # attention is queued — its long dependency chain (denom DMA →
            # reciprocal → rrow DMA → bcast → multiply) then resolves while
            # the PE works on attention instead of stalling its FIFO.
            # Global schedule: projections interleave with chunks of already-
            # projected pairs so ScalarE never starves; each chunk's
            # normalize (DVE/DMA-only) fires once its 4 pairs are done, and
            # merge matmuls spread between later attention chunks.
            sched = [
                "p0", "A00", "p1", "A01", "A10", "p2", "A11", "A02",
                "A20", "p3", "A21", "A12", "A30", "A03", "n0", "A31",
                "m00", "m01", "A22", "m02", "m03", "A13", "n1", "A32",
                "m10", "m11", "A23", "n2", "m12", "m13", "A33", "m20",
                "m21", "m22", "m23", "n3", "m30", "m31", "m32", "m33",
            ]
            for step in sched:
                kind, a = step[0], step[1:]
                if kind == "p":
                    project_pair(int(a))
                elif kind == "A":
                    attn_chunk(int(a[1]), int(a[0]))
                elif kind == "n":
                    norm_chunk(int(a))
                elif kind == "m":
                    merge_mo(int(a[0]), int(a[1]))

    _split_sync_waits(nc)
    return nc


_NC_CACHE = None
LAST_RESULT = None


def _get_nc():
    global _NC_CACHE
    if _NC_CACHE is None:
        _NC_CACHE = _build_program()
    return _NC_CACHE


def _host_prep(query, key, value, enc_cos, enc_sin, wq, bq, wk, bk, wv, bv, wm, bm):
    """Host-side weight permutation / transposition / dtype casts."""
    c = np.arange(D)
    perm = (c % DH) * H + (c // DH)  # new head-contiguous channel -> original

    wq_r, wk_r, wv_r = wq[perm], wk[perm], wv[perm]
    bq_r, bk_r, bv_r = bq[perm], bk[perm], bv[perm]
    wm_r = wm[:, perm]

    r64 = np.zeros((DH, DH), np.float32)
    idx = np.arange(DH // 2)
    r64[2 * idx, 2 * idx + 1] = -1.0
    r64[2 * idx + 1, 2 * idx] = 1.0
    r128 = np.zeros((128, 128), np.float32)
    r128[0:64, 0:64] = r64
    r128[64:128, 64:128] = r64

    cos64 = enc_cos[0, :, 0, :]
    sin64 = enc_sin[0, :, 0, :]

    shared = {
        "wqT": np.ascontiguousarray(wq_r.T).astype(np.float16),
        "wkT": np.ascontiguousarray(wk_r.T).astype(np.float16),
        "wvT": np.ascontiguousarray(wv_r.T).astype(np.float16),
        "wmT": np.ascontiguousarray(wm_r.T).astype(np.float16),
        "rt": np.ascontiguousarray(r128.T).astype(np.float16),
        "cos2": np.vstack([cos64, cos64]).astype(np.float16),
        "sin2": np.vstack([sin64, sin64]).astype(np.float16),
        "bq": np.ascontiguousarray(bq_r.reshape(4, 128).T).astype(np.float32),
        "bk": np.ascontiguousarray(bk_r.reshape(4, 128).T).astype(np.float32),
        "bm": np.ascontiguousarray(bm.reshape(4, 128).T).astype(np.float32),
        "bv_row": bv_r[None, :].astype(np.float16),
    }
    q16 = np.asarray(query, np.float16)
    k16 = np.asarray(key, np.float16)
    v16 = np.asarray(value, np.float16)
    in_maps = []
    for b in range(B):
        m = dict(shared)
        m["xq"] = np.ascontiguousarray(q16[b])
        m["xk"] = np.ascontiguousarray(k16[b])
        m["xv"] = np.ascontiguousarray(v16[b])
        in_maps.append(m)
    return in_maps


def kernel(query, key, value, enc_cos, enc_sin, wq, bq, wk, bk, wv, bv, wm, bm):
    global LAST_RESULT
    query = np.asarray(query, np.float32)
    key = np.asarray(key, np.float32)
    value = np.asarray(value, np.float32)
    in_maps = _host_prep(
        query, key, value,
        np.asarray(enc_cos, np.float32), np.asarray(enc_sin, np.float32),
        np.asarray(wq, np.float32), np.asarray(bq, np.float32),
        np.asarray(wk, np.float32), np.asarray(bk, np.float32),
        np.asarray(wv, np.float32), np.asarray(bv, np.float32),
        np.asarray(wm, np.float32), np.asarray(bm, np.float32),
    )
    nc = _get_nc()
    res = run_bass_kernel_spmd(nc, in_maps, core_ids=list(range(NCORES)))
    LAST_RESULT = res
    out = np.stack([res.results[b]["out"] for b in range(B)], axis=0)
    return out.astype(np.float32)
